# revision 44
# baseline (speedup 1.0000x reference)
"""Trainium2 Bass kernel for causal GQA self-attention (fused QKV + RoPE).

Problem: B=2, T=2048, C=2048, H=16 q-heads, KV=4 kv-heads, HD=128.
Sharding: 8 cores = (batch b, kv-group k). Each core computes the 4 q-heads
of one kv group for one batch element; outputs are disjoint slices of y.

Per-core device kernel (bf16 matmuls, fp32 PSUM accumulation):
  1. QKV projection qkv^T = W_shard @ x^T, d-major layout [j, t].
     Host pre-transposes x and W (and pre-permutes q/k head dims so RoPE
     becomes rotate-half instead of interleaved pairs), casting to bf16.
     Weight quarters are DMA'd just-in-time; the first (wt, xt) pair is
     split per c-chunk so the first matmul starts ~4us in.
  2. RoPE on q/k via SBUF->SBUF DMA partition swap + DVE mul/add (bf16).
  3. Attention in S^T orientation: scores^T[s,t] = K^T.T @ Q^T per
     (128 s-chunk x 512 t-block), exp on ScalarE (bf16 out), causal diagonal
     masked by a triangular multiply. Row sums: exp chunks are combined in
     groups of 4 on GpSimd+DVE, then one all-ones stationary matmul per
     group (4x fewer PE columns than per-chunk sums). PV accumulates
     unnormalized y^T[d,t] in PSUM (fp32).
  4. Projection tt=k is software-pipelined with attention t-block k-1 so
     ScalarE exp work hides under projection matmuls. PSUM banks are
     partitioned by tag: proj accumulators 3, attention y/sum 2, scores 2
     (+1 spare), so neither side can starve the other. In the last t-block
     (attention only, ScalarE-bound) score chunks are paired into 2-bank
     PSUM tiles and exp'd with one 1024-wide ACT to halve ACT overhead.
Output per core: unnormalized y^T [512, 2048] + row sums [16, 512]; the host
divides, transposes and concatenates.
"""

import math

import numpy as np
import ml_dtypes

import concourse.bass as bass
import concourse.mybir as mybir
import concourse.tile as tile
from concourse import bacc
from concourse.bass_utils import run_bass_kernel_spmd

B, T, C = 2, 2048, 2048
H, KV, HD = 16, 4, 128
NREP = H // KV  # q heads per core
P = 128
NCORES = 8
CC_CHUNKS = C // P  # 16 contraction chunks
TT = 4  # t-blocks of 512
TB = T // TT  # 512
NB = 6  # j-blocks per core: q0..q3, k, v
SCALE = 1.0 / math.sqrt(HD)

f32 = mybir.dt.float32
bf16 = mybir.dt.bfloat16

TRACE = False  # set True (with ntff shim installed) to get exec_time_ns

_cache = {}


def _build():
    if "nc" in _cache:
        return _cache["nc"]

    nc = bacc.Bacc("TRN2", target_bir_lowering=False, debug=False,
                   num_devices=NCORES)

    # x^T in per-(cq, tt) contiguous blocks: [P, cq, tt, ci, TB]
    xT_d = nc.dram_tensor("xT", [P, 4, TT, 4, TB], bf16,
                          kind="ExternalInput").ap()
    # W^T in per-quarter contiguous blocks: [P, wq, ci, NB*P]
    wT_d = nc.dram_tensor("wT", [P, 4, 4, NB * P], bf16,
                          kind="ExternalInput").ap()
    cc_d = nc.dram_tensor("CC", [P, T], bf16, kind="ExternalInput").ap()
    ss_d = nc.dram_tensor("SS2", [P, T], bf16, kind="ExternalInput").ap()
    tri_d = nc.dram_tensor("tri", [P, P], bf16, kind="ExternalInput").ap()
    ones_d = nc.dram_tensor("ones", [P, P], bf16, kind="ExternalInput").ap()
    ident_d = nc.dram_tensor("ident", [P, P], bf16, kind="ExternalInput").ap()
    yT_d = nc.dram_tensor("yT", [NREP * P, T], bf16, kind="ExternalOutput").ap()
    sums_d = nc.dram_tensor("sums", [NREP * TT, TB], f32, kind="ExternalOutput").ap()

    with tile.TileContext(nc) as tc:
        with (
            tc.tile_pool(name="wt", bufs=1) as wt_pool,
            tc.tile_pool(name="xt", bufs=5) as xt_pool,
            tc.tile_pool(name="qkvt", bufs=1) as qkv_pool,
            tc.tile_pool(name="freq", bufs=1) as freq_pool,
            tc.tile_pool(name="small", bufs=1) as small_pool,
            tc.tile_pool(name="vsb", bufs=1) as v_pool,
            tc.tile_pool(name="swp", bufs=2) as swp_pool,
            tc.tile_pool(name="ropetmp", bufs=2) as rt_pool,
            tc.tile_pool(name="expt", bufs=10) as exp_pool,
            tc.tile_pool(name="ecum", bufs=2) as ec_pool,
            tc.tile_pool(name="yout", bufs=2) as y_pool,
            tc.tile_pool(name="psproj", bufs=3, space="PSUM") as psproj_pool,
            tc.tile_pool(name="psacc", bufs=2, space="PSUM") as psacc_pool,
            tc.tile_pool(name="pss", bufs=3, space="PSUM") as pss_pool,
        ):
            # ---- resident tensors ----
            wt_q = [
                wt_pool.tile([P, 4, NB * P], bf16, tag=f"wt{wq}", name=f"wt{wq}")
                for wq in range(4)
            ]
            wt_loaded = [False] * 4

            # qkv^T blocks [128 d, 2048 t]: jb 0..3 = q heads (rope-permuted),
            # 4 = k (rope-permuted), 5 = v
            qkvT = [
                qkv_pool.tile([P, T], bf16, tag=f"qkv{jb}", name=f"qkv{jb}")
                for jb in range(NB)
            ]
            # V in s-major: [128 s, 16 s-chunk, 128 d]
            v_sb = v_pool.tile([P, CC_CHUNKS, P], bf16, tag="vsb")

            # late-loaded constants (needed only after proj tt0 matmuls)
            ccs = freq_pool.tile([P, T], bf16, tag="cc")
            ss2 = freq_pool.tile([P, T], bf16, tag="ss")
            tri = small_pool.tile([P, P], bf16, tag="tri")
            ones = small_pool.tile([P, P], bf16, tag="ones")
            ident = small_pool.tile([P, P], bf16, tag="ident")
            _late = [(ccs, cc_d), (ss2, ss_d), (tri, tri_d), (ones, ones_d),
                     (ident, ident_d)]

            # xt tiles for a tt block (loaded during proj group A, reused by B)
            xt_tiles = {}

            def proj_mms(tt, jbs, psums):
                """Projection matmuls for j-blocks `jbs` of t-block tt."""
                for cq in range(4):
                    finegrain = tt == 0
                    key = (tt, cq)
                    if finegrain and not wt_loaded[cq]:
                        # supply-latency critical region: alternate w/x
                        # per c-chunk on the two HWDGE queues (sync + scalar)
                        xt = xt_pool.tile([P, 4, TB], bf16, tag="xt", name="xt")
                        for ci in range(4):
                            # balance bytes across the two HWDGE queues
                            qa, qb = ((nc.sync, nc.scalar) if (cq + ci) % 2 == 0
                                      else (nc.scalar, nc.sync))
                            qa.dma_start(
                                wt_q[cq][:, ci:ci + 1, :],
                                wT_d[:, cq, ci:ci + 1, :],
                            )
                            qb.dma_start(
                                xt[:, ci:ci + 1, :],
                                xT_d[:, cq, 0, ci:ci + 1, :],
                            )
                        wt_loaded[cq] = True
                        xt_tiles[key] = xt
                        if cq == 1:
                            # rope/mask constants: needed first by rope(tt0)
                            # at ~25us; split across queues behind cq0/cq1
                            for n_, (_tile, _src) in enumerate(_late):
                                eng = nc.sync if n_ % 2 == 0 else nc.scalar
                                eng.dma_start(_tile[:], _src[:])
                            _late.clear()
                    else:
                        if not wt_loaded[cq]:
                            nc.sync.dma_start(wt_q[cq][:], wT_d[:, cq])
                            wt_loaded[cq] = True
                        if key not in xt_tiles:
                            xt = xt_pool.tile([P, 4, TB], bf16, tag="xt",
                                              name="xt")
                            nc.sync.dma_start(xt[:], xT_d[:, cq, tt])
                            xt_tiles[key] = xt
                    xt = xt_tiles[key]
                    for ci in range(4):
                        cc = cq * 4 + ci
                        for j, jb in enumerate(jbs):
                            nc.tensor.matmul(
                                psums[j][:],
                                wt_q[cq][:, ci, jb * P:(jb + 1) * P],
                                xt[:, ci, :],
                                start=(cc == 0),
                                stop=(cc == CC_CHUNKS - 1),
                            )

            def proj_post(tt, jbs, psums):
                """PSUM->SBUF copies, RoPE (q/k) and V transpose for `jbs`."""
                tsl = slice(tt * TB, (tt + 1) * TB)
                for j, jb in enumerate(jbs):
                    if jb % 2 == 0:
                        nc.vector.tensor_copy(qkvT[jb][:, tsl], psums[j][:])
                    else:
                        nc.scalar.copy(qkvT[jb][:, tsl], psums[j][:])
                for jb in jbs:
                    if jb == 5:
                        # V transpose for this chunk: v^T [d, s] -> v_sb [s, d]
                        for i in range(4):
                            sc = 4 * tt + i
                            trp = psproj_pool.tile([P, TB], bf16, tag="proj",
                                                   name="trp")
                            nc.tensor.transpose(
                                trp[:, :P], qkvT[5][:, sc * P:(sc + 1) * P],
                                ident[:]
                            )
                            nc.vector.tensor_copy(v_sb[:, sc, :], trp[:, :P])
                    else:
                        # RoPE on this t-chunk
                        swp = swp_pool.tile([P, TB], bf16, tag="swp", name="swp")
                        nc.sync.dma_start(swp[0:64, :], qkvT[jb][64:128, tsl])
                        nc.sync.dma_start(swp[64:128, :], qkvT[jb][0:64, tsl])
                        ta = rt_pool.tile([P, TB], bf16, tag="ta", name="ta")
                        tb_ = rt_pool.tile([P, TB], bf16, tag="tb", name="tb")
                        nc.vector.tensor_tensor(
                            ta[:], qkvT[jb][:, tsl], ccs[:, tsl],
                            mybir.AluOpType.mult
                        )
                        nc.vector.tensor_tensor(
                            tb_[:], swp[:], ss2[:, tsl], mybir.AluOpType.mult
                        )
                        nc.vector.tensor_tensor(
                            qkvT[jb][:, tsl], ta[:], tb_[:], mybir.AluOpType.add
                        )

            def attn_head(tb, h, pair):
                """Attention for (t-block tb, head h), S^T orientation.

                pair=True: full (non-diagonal) chunk groups compute scores for
                2 chunks into one 2-bank PSUM tile and exp them with a single
                1024-wide ACT (halves ACT instruction overhead)."""
                psum_y = psacc_pool.tile([P, TB], f32, tag="acc", name="psum_y")
                psum_sum = psacc_pool.tile([P, TB], f32, tag="acc",
                                           name="psum_sum")
                nsc = 4 * (tb + 1)
                ngroups = tb + 1
                for g in range(ngroups):
                    diag = g == tb
                    etiles = []
                    if True:
                        for i4 in range(4):
                            sc = 4 * g + i4
                            r = sc - 4 * tb  # >=0: diagonal-crossing chunk
                            col0 = r * P if r >= 0 else 0
                            psum_s = pss_pool.tile([P, TB], f32, tag="s",
                                                   name="psum_s")
                            nc.tensor.matmul(
                                psum_s[:, col0:],
                                qkvT[4][:, sc * P:(sc + 1) * P],
                                qkvT[h][:, tb * TB + col0:(tb + 1) * TB],
                                start=True,
                                stop=True,
                            )
                            expt = exp_pool.tile([P, TB], bf16, tag="expt",
                                                 name="expt")
                            nc.scalar.activation(
                                expt[:, col0:],
                                psum_s[:, col0:],
                                mybir.ActivationFunctionType.Exp,
                                scale=SCALE,
                            )
                            if r >= 0:
                                nc.vector.tensor_tensor(
                                    expt[:, col0:col0 + P],
                                    expt[:, col0:col0 + P],
                                    tri[:],
                                    mybir.AluOpType.mult,
                                )
                            nc.tensor.matmul(
                                psum_y[:, col0:],
                                v_sb[:, sc, :],
                                expt[:, col0:],
                                start=(sc == 0),
                                stop=(sc == nsc - 1),
                            )
                            etiles.append(expt[:])
                    # combine the group's 4 exp chunks, then one sum matmul
                    ec = ec_pool.tile([P, TB], bf16, tag="ec", name="ec")
                    e0, e1, e2_, e3 = etiles
                    if not diag:
                        # full chunks; pairwise tree split across GpSimd
                        # (otherwise idle) and DVE
                        ec2 = ec_pool.tile([P, TB], bf16, tag="ec2", name="ec2")
                        nc.vector.tensor_tensor(
                            ec[:], e0, e1, mybir.AluOpType.add
                        )
                        nc.vector.tensor_tensor(
                            ec2[:], e2_, e3, mybir.AluOpType.add
                        )
                        nc.vector.tensor_tensor(
                            ec[:], ec[:], ec2[:], mybir.AluOpType.add
                        )
                    else:
                        # r = 0..3 trims: valid cols are [r*P:]
                        nc.vector.tensor_copy(ec[:, 0:P], e0[:, 0:P])
                        nc.vector.tensor_tensor(
                            ec[:, P:], e0[:, P:], e1[:, P:], mybir.AluOpType.add
                        )
                        nc.vector.tensor_tensor(
                            ec[:, 2 * P:], ec[:, 2 * P:], e2_[:, 2 * P:],
                            mybir.AluOpType.add
                        )
                        nc.vector.tensor_tensor(
                            ec[:, 3 * P:], ec[:, 3 * P:], e3[:, 3 * P:],
                            mybir.AluOpType.add
                        )
                    nc.tensor.matmul(
                        psum_sum[:],
                        ones[:],
                        ec[:],
                        start=(g == 0),
                        stop=(g == ngroups - 1),
                    )
                y_sb = y_pool.tile([P, TB], bf16, tag="ysb", name="ysb")
                if tb == TT - 1 and h == NREP - 1:
                    # tail: split the last writeout so DMA overlaps the copy
                    for q in range(2):
                        hsl = slice(q * (TB // 2), (q + 1) * (TB // 2))
                        nc.vector.tensor_copy(y_sb[:, hsl], psum_y[:, hsl])
                        nc.sync.dma_start(
                            yT_d[h * P:(h + 1) * P,
                                 tb * TB + q * (TB // 2):
                                 tb * TB + (q + 1) * (TB // 2)],
                            y_sb[:, hsl],
                        )
                else:
                    nc.vector.tensor_copy(y_sb[:], psum_y[:])
                    nc.sync.dma_start(
                        yT_d[h * P:(h + 1) * P, tb * TB:(tb + 1) * TB], y_sb[:]
                    )
                sums_sb = y_pool.tile([1, TB], f32, tag="sums", name="sums_sb")
                nc.vector.tensor_copy(sums_sb[:], psum_sum[0:1, :])
                nc.sync.dma_start(
                    sums_d[h * TT + tb:h * TT + tb + 1, :], sums_sb[0:1, :]
                )

            # ---- software pipeline: proj(tt) overlapped with attn(tb=tt-1) ----
            GA, GB = [0, 1, 2], [3, 4, 5]
            # step 0: no attention to overlap, so run all 6 j-blocks at once
            # (borrowing the idle attention PSUM banks) — this halves the
            # DMA demand rate per unit of PE work and avoids supply stalls
            ps0 = (
                [psproj_pool.tile([P, TB], f32, tag="proj", name="proj_ps")
                 for _ in range(3)]
                + [psacc_pool.tile([P, TB], f32, tag="acc", name="proj_ps")
                   for _ in range(2)]
                + [pss_pool.tile([P, TB], f32, tag="s", name="proj_ps")]
            )
            proj_mms(0, GA + GB, ps0)
            proj_post(0, GA + GB, ps0)
            for cq in range(4):
                xt_tiles.pop((0, cq), None)
            for step in range(1, TT + 1):
                tt = step if step < TT else None
                tb = step - 1
                attn_head(tb, 0, False)
                attn_head(tb, 1, False)
                if tt is not None:
                    psA = [psproj_pool.tile([P, TB], f32, tag="proj",
                                            name="proj_ps") for _ in GA]
                    proj_mms(tt, GA, psA)
                attn_head(tb, 2, False)
                if tt is not None:
                    proj_post(tt, GA, psA)
                attn_head(tb, 3, False)
                if tt is not None:
                    psB = [psproj_pool.tile([P, TB], f32, tag="proj",
                                            name="proj_ps") for _ in GB]
                    proj_mms(tt, GB, psB)
                    proj_post(tt, GB, psB)
                    # xt tiles for this tt are dead now
                    for cq in range(4):
                        xt_tiles.pop((tt, cq), None)

    nc.compile()
    _cache["nc"] = nc
    return nc


def _host_prep(x, w_qkv, freqs_cos, freqs_sin):
    """Build per-core input maps (numpy, cheap)."""
    x = np.asarray(x, dtype=np.float32)
    w_qkv = np.asarray(w_qkv, dtype=np.float32)
    freqs_cos = np.asarray(freqs_cos, dtype=np.float32)
    freqs_sin = np.asarray(freqs_sin, dtype=np.float32)
    bf = ml_dtypes.bfloat16

    perm = np.concatenate([np.arange(0, HD, 2), np.arange(1, HD, 2)])

    xTs = []
    for b in range(B):
        # [C, T] -> [P, cq, tt, ci, TB]
        xt = x[b].T.reshape(4, 4, P, TT, TB).transpose(2, 0, 3, 1, 4)
        xTs.append(np.ascontiguousarray(xt).astype(bf))

    cosT = freqs_cos.T  # [64, T]
    sinT = freqs_sin.T
    CCh = np.concatenate([cosT, cosT], axis=0).astype(bf)
    SS2 = np.concatenate([-sinT, sinT], axis=0).astype(bf)
    tri = np.triu(np.ones((P, P), dtype=np.float32)).astype(bf)
    ones = np.ones((P, P), dtype=bf)
    ident = np.eye(P, dtype=np.float32).astype(bf)

    in_maps = []
    for core in range(NCORES):
        b, kv = divmod(core, KV)
        blocks = []
        for r in range(NREP):
            hrow = (kv * NREP + r) * HD
            blocks.append(w_qkv[hrow:hrow + HD][perm])
        blocks.append(w_qkv[H * HD + kv * HD:H * HD + (kv + 1) * HD][perm])
        blocks.append(
            w_qkv[(H + KV) * HD + kv * HD:(H + KV) * HD + (kv + 1) * HD]
        )
        w_shard = np.concatenate(blocks, axis=0)  # [768, C]
        # [C, 768] -> [P, wq, ci, 768]
        wT = w_shard.T.reshape(4, 4, P, NB * P).transpose(2, 0, 1, 3)
        wT = np.ascontiguousarray(wT).astype(bf)
        in_maps.append({
            "xT": xTs[b],
            "wT": wT,
            "CC": CCh,
            "SS2": SS2,
            "tri": tri,
            "ones": ones,
            "ident": ident,
        })
    return in_maps


def kernel(x, w_qkv, freqs_cos, freqs_sin):
    nc = _build()
    in_maps = _host_prep(x, w_qkv, freqs_cos, freqs_sin)
    res = run_bass_kernel_spmd(nc, in_maps, list(range(NCORES)), trace=TRACE)
    _cache["last_res"] = res

    y = np.empty((B, T, C), dtype=np.float32)
    for core in range(NCORES):
        b, kv = divmod(core, KV)
        yT = res.results[core]["yT"].astype(np.float32)  # [NREP*P, T] unnorm.
        sums = res.results[core]["sums"].reshape(NREP, T)  # per (h, t)
        yT = yT.reshape(NREP, P, T) / sums[:, None, :]
        y[b, :, kv * NREP * HD:(kv + 1) * NREP * HD] = (
            yT.reshape(NREP * P, T).T
        )
    return y


# revision 45
# speedup vs baseline: 1.0222x; 1.0222x over previous
"""Trainium2 Bass kernel for causal GQA self-attention (fused QKV + RoPE).

Problem: B=2, T=2048, C=2048, H=16 q-heads, KV=4 kv-heads, HD=128.
Sharding: 8 cores = (batch b, kv-group k). Each core computes the 4 q-heads
of one kv group for one batch element; outputs are disjoint slices of y.

Per-core device kernel (bf16 matmuls, fp32 PSUM accumulation):
  1. QKV projection qkv^T = W_shard @ x^T, d-major layout [j, t].
     Host pre-transposes x and W (and pre-permutes q/k head dims so RoPE
     becomes rotate-half instead of interleaved pairs), casting to bf16.
     Weight quarters are DMA'd just-in-time; the first (wt, xt) pair is
     split per c-chunk so the first matmul starts ~4us in.
  2. RoPE on q/k via SBUF->SBUF DMA partition swap + DVE mul/add (bf16).
  3. Attention in S^T orientation: scores^T[s,t] = K^T.T @ Q^T per
     (128 s-chunk x 512 t-block), exp on ScalarE (bf16 out), causal diagonal
     masked by a triangular multiply. Row sums: exp chunks are combined in
     groups of 4 on GpSimd+DVE, then one all-ones stationary matmul per
     group (4x fewer PE columns than per-chunk sums). PV accumulates
     unnormalized y^T[d,t] in PSUM (fp32).
  4. Projection tt=k is software-pipelined with attention t-block k-1 so
     ScalarE exp work hides under projection matmuls. PSUM banks are
     partitioned by tag: proj accumulators 3, attention y/sum 2, scores 2
     (+1 spare), so neither side can starve the other. In the last t-block
     (attention only, ScalarE-bound) score chunks are paired into 2-bank
     PSUM tiles and exp'd with one 1024-wide ACT to halve ACT overhead.
Output per core: unnormalized y^T [512, 2048] + row sums [16, 512]; the host
divides, transposes and concatenates.
"""

import math

import numpy as np
import ml_dtypes

import concourse.bass as bass
import concourse.mybir as mybir
import concourse.tile as tile
from concourse import bacc
from concourse.bass_utils import run_bass_kernel_spmd

B, T, C = 2, 2048, 2048
H, KV, HD = 16, 4, 128
NREP = H // KV  # q heads per core
P = 128
NCORES = 8
CC_CHUNKS = C // P  # 16 contraction chunks
TT = 4  # t-blocks of 512
TB = T // TT  # 512
NB = 6  # j-blocks per core: q0..q3, k, v
SCALE = 1.0 / math.sqrt(HD)

f32 = mybir.dt.float32
bf16 = mybir.dt.bfloat16

TRACE = False  # set True (with ntff shim installed) to get exec_time_ns

_cache = {}


def _build():
    if "nc" in _cache:
        return _cache["nc"]

    nc = bacc.Bacc("TRN2", target_bir_lowering=False, debug=False,
                   num_devices=NCORES)

    # x^T in per-(cq, tt) contiguous blocks: [P, cq, tt, ci, TB]
    xT_d = nc.dram_tensor("xT", [P, 4, TT, 4, TB], bf16,
                          kind="ExternalInput").ap()
    # W^T in per-quarter contiguous blocks: [P, wq, ci, NB*P]
    wT_d = nc.dram_tensor("wT", [P, 4, 4, NB * P], bf16,
                          kind="ExternalInput").ap()
    cc_d = nc.dram_tensor("CC", [P, T], bf16, kind="ExternalInput").ap()
    ss_d = nc.dram_tensor("SS2", [P, T], bf16, kind="ExternalInput").ap()
    tri_d = nc.dram_tensor("tri", [P, P], bf16, kind="ExternalInput").ap()
    ones_d = nc.dram_tensor("ones", [P, P], bf16, kind="ExternalInput").ap()
    ident_d = nc.dram_tensor("ident", [P, P], bf16, kind="ExternalInput").ap()
    yT_d = nc.dram_tensor("yT", [NREP * P, T], bf16, kind="ExternalOutput").ap()
    sums_d = nc.dram_tensor("sums", [NREP * TT, TB], f32, kind="ExternalOutput").ap()

    with tile.TileContext(nc) as tc:
        with (
            tc.tile_pool(name="wt", bufs=1) as wt_pool,
            tc.tile_pool(name="xt", bufs=5) as xt_pool,
            tc.tile_pool(name="qkvt", bufs=1) as qkv_pool,
            tc.tile_pool(name="freq", bufs=1) as freq_pool,
            tc.tile_pool(name="small", bufs=1) as small_pool,
            tc.tile_pool(name="vsb", bufs=1) as v_pool,
            tc.tile_pool(name="swp", bufs=2) as swp_pool,
            tc.tile_pool(name="ropetmp", bufs=2) as rt_pool,
            tc.tile_pool(name="expt", bufs=10) as exp_pool,
            tc.tile_pool(name="ecum", bufs=2) as ec_pool,
            tc.tile_pool(name="yout", bufs=2) as y_pool,
            tc.tile_pool(name="psproj", bufs=3, space="PSUM") as psproj_pool,
            tc.tile_pool(name="psacc", bufs=2, space="PSUM") as psacc_pool,
            tc.tile_pool(name="pss", bufs=3, space="PSUM") as pss_pool,
        ):
            # ---- resident tensors ----
            wt_q = [
                wt_pool.tile([P, 4, NB * P], bf16, tag=f"wt{wq}", name=f"wt{wq}")
                for wq in range(4)
            ]
            wt_loaded = [False] * 4

            # qkv^T blocks [128 d, 2048 t]: jb 0..3 = q heads (rope-permuted),
            # 4 = k (rope-permuted), 5 = v
            qkvT = [
                qkv_pool.tile([P, T], bf16, tag=f"qkv{jb}", name=f"qkv{jb}")
                for jb in range(NB)
            ]
            # V in s-major: [128 s, 16 s-chunk, 128 d]
            v_sb = v_pool.tile([P, CC_CHUNKS, P], bf16, tag="vsb")

            # late-loaded constants (needed only after proj tt0 matmuls)
            ccs = freq_pool.tile([P, T], bf16, tag="cc")
            ss2 = freq_pool.tile([P, T], bf16, tag="ss")
            tri = small_pool.tile([P, P], bf16, tag="tri")
            ones = small_pool.tile([P, P], bf16, tag="ones")
            ident = small_pool.tile([P, P], bf16, tag="ident")
            _late = [(ccs, cc_d), (ss2, ss_d), (tri, tri_d), (ones, ones_d),
                     (ident, ident_d)]

            # xt tiles for a tt block (loaded during proj group A, reused by B)
            xt_tiles = {}

            def proj_mms(tt, jbs, psums):
                """Projection matmuls for j-blocks `jbs` of t-block tt."""
                for cq in range(4):
                    finegrain = tt == 0
                    key = (tt, cq)
                    if finegrain and not wt_loaded[cq]:
                        # supply-latency critical region: alternate w/x
                        # per c-chunk on the two HWDGE queues (sync + scalar)
                        xt = xt_pool.tile([P, 4, TB], bf16, tag="xt", name="xt")
                        for ci in range(4):
                            # balance bytes across the two HWDGE queues
                            qa, qb = ((nc.sync, nc.scalar) if (cq + ci) % 2 == 0
                                      else (nc.scalar, nc.sync))
                            qa.dma_start(
                                wt_q[cq][:, ci:ci + 1, :],
                                wT_d[:, cq, ci:ci + 1, :],
                            )
                            qb.dma_start(
                                xt[:, ci:ci + 1, :],
                                xT_d[:, cq, 0, ci:ci + 1, :],
                            )
                        wt_loaded[cq] = True
                        xt_tiles[key] = xt
                        if cq == 1:
                            # rope/mask constants: needed first by rope(tt0)
                            # at ~25us; split across queues behind cq0/cq1
                            for n_, (_tile, _src) in enumerate(_late):
                                eng = nc.sync if n_ % 2 == 0 else nc.scalar
                                eng.dma_start(_tile[:], _src[:])
                            _late.clear()
                    else:
                        if not wt_loaded[cq]:
                            nc.sync.dma_start(wt_q[cq][:], wT_d[:, cq])
                            wt_loaded[cq] = True
                        if key not in xt_tiles:
                            xt = xt_pool.tile([P, 4, TB], bf16, tag="xt",
                                              name="xt")
                            nc.sync.dma_start(xt[:], xT_d[:, cq, tt])
                            xt_tiles[key] = xt
                    xt = xt_tiles[key]
                    for ci in range(4):
                        cc = cq * 4 + ci
                        for j, jb in enumerate(jbs):
                            nc.tensor.matmul(
                                psums[j][:],
                                wt_q[cq][:, ci, jb * P:(jb + 1) * P],
                                xt[:, ci, :],
                                start=(cc == 0),
                                stop=(cc == CC_CHUNKS - 1),
                            )

            def proj_post(tt, jbs, psums):
                """PSUM->SBUF copies, RoPE (q/k) and V transpose for `jbs`."""
                tsl = slice(tt * TB, (tt + 1) * TB)
                for j, jb in enumerate(jbs):
                    if jb % 2 == 0:
                        nc.vector.tensor_copy(qkvT[jb][:, tsl], psums[j][:])
                    else:
                        nc.scalar.copy(qkvT[jb][:, tsl], psums[j][:])
                for jb in jbs:
                    if jb == 5:
                        # V transpose for this chunk: v^T [d, s] -> v_sb [s, d]
                        for i in range(4):
                            sc = 4 * tt + i
                            trp = pss_pool.tile([P, TB], bf16, tag="s",
                                                name="trp")
                            nc.tensor.transpose(
                                trp[:, :P], qkvT[5][:, sc * P:(sc + 1) * P],
                                ident[:]
                            )
                            nc.vector.tensor_copy(v_sb[:, sc, :], trp[:, :P])
                    else:
                        # RoPE on this t-chunk
                        swp = swp_pool.tile([P, TB], bf16, tag="swp", name="swp")
                        nc.sync.dma_start(swp[0:64, :], qkvT[jb][64:128, tsl])
                        nc.sync.dma_start(swp[64:128, :], qkvT[jb][0:64, tsl])
                        ta = rt_pool.tile([P, TB], bf16, tag="ta", name="ta")
                        tb_ = rt_pool.tile([P, TB], bf16, tag="tb", name="tb")
                        nc.vector.tensor_tensor(
                            ta[:], qkvT[jb][:, tsl], ccs[:, tsl],
                            mybir.AluOpType.mult
                        )
                        nc.vector.tensor_tensor(
                            tb_[:], swp[:], ss2[:, tsl], mybir.AluOpType.mult
                        )
                        nc.vector.tensor_tensor(
                            qkvT[jb][:, tsl], ta[:], tb_[:], mybir.AluOpType.add
                        )

            def attn_head(tb, h, pair):
                """Attention for (t-block tb, head h), S^T orientation.

                pair=True: full (non-diagonal) chunk groups compute scores for
                2 chunks into one 2-bank PSUM tile and exp them with a single
                1024-wide ACT (halves ACT instruction overhead)."""
                psum_y = psacc_pool.tile([P, TB], f32, tag="acc", name="psum_y")
                psum_sum = psacc_pool.tile([P, TB], f32, tag="acc",
                                           name="psum_sum")
                nsc = 4 * (tb + 1)
                ngroups = tb + 1
                for g in range(ngroups):
                    diag = g == tb
                    etiles = []
                    if True:
                        for i4 in range(4):
                            sc = 4 * g + i4
                            r = sc - 4 * tb  # >=0: diagonal-crossing chunk
                            col0 = r * P if r >= 0 else 0
                            psum_s = pss_pool.tile([P, TB], f32, tag="s",
                                                   name="psum_s")
                            nc.tensor.matmul(
                                psum_s[:, col0:],
                                qkvT[4][:, sc * P:(sc + 1) * P],
                                qkvT[h][:, tb * TB + col0:(tb + 1) * TB],
                                start=True,
                                stop=True,
                            )
                            expt = exp_pool.tile([P, TB], bf16, tag="expt",
                                                 name="expt")
                            nc.scalar.activation(
                                expt[:, col0:],
                                psum_s[:, col0:],
                                mybir.ActivationFunctionType.Exp,
                                scale=SCALE,
                            )
                            if r >= 0:
                                nc.vector.tensor_tensor(
                                    expt[:, col0:col0 + P],
                                    expt[:, col0:col0 + P],
                                    tri[:],
                                    mybir.AluOpType.mult,
                                )
                            nc.tensor.matmul(
                                psum_y[:, col0:],
                                v_sb[:, sc, :],
                                expt[:, col0:],
                                start=(sc == 0),
                                stop=(sc == nsc - 1),
                            )
                            etiles.append(expt[:])
                    # combine the group's 4 exp chunks, then one sum matmul
                    ec = ec_pool.tile([P, TB], bf16, tag="ec", name="ec")
                    e0, e1, e2_, e3 = etiles
                    if not diag:
                        # full chunks; pairwise tree split across GpSimd
                        # (otherwise idle) and DVE
                        ec2 = ec_pool.tile([P, TB], bf16, tag="ec2", name="ec2")
                        nc.vector.tensor_tensor(
                            ec[:], e0, e1, mybir.AluOpType.add
                        )
                        nc.vector.tensor_tensor(
                            ec2[:], e2_, e3, mybir.AluOpType.add
                        )
                        nc.vector.tensor_tensor(
                            ec[:], ec[:], ec2[:], mybir.AluOpType.add
                        )
                    else:
                        # r = 0..3 trims: valid cols are [r*P:]
                        nc.vector.tensor_copy(ec[:, 0:P], e0[:, 0:P])
                        nc.vector.tensor_tensor(
                            ec[:, P:], e0[:, P:], e1[:, P:], mybir.AluOpType.add
                        )
                        nc.vector.tensor_tensor(
                            ec[:, 2 * P:], ec[:, 2 * P:], e2_[:, 2 * P:],
                            mybir.AluOpType.add
                        )
                        nc.vector.tensor_tensor(
                            ec[:, 3 * P:], ec[:, 3 * P:], e3[:, 3 * P:],
                            mybir.AluOpType.add
                        )
                    nc.tensor.matmul(
                        psum_sum[:],
                        ones[:],
                        ec[:],
                        start=(g == 0),
                        stop=(g == ngroups - 1),
                    )
                y_sb = y_pool.tile([P, TB], bf16, tag="ysb", name="ysb")
                if tb == TT - 1 and h == NREP - 1:
                    # tail: split the last writeout so DMA overlaps the copy
                    for q in range(2):
                        hsl = slice(q * (TB // 2), (q + 1) * (TB // 2))
                        nc.vector.tensor_copy(y_sb[:, hsl], psum_y[:, hsl])
                        nc.sync.dma_start(
                            yT_d[h * P:(h + 1) * P,
                                 tb * TB + q * (TB // 2):
                                 tb * TB + (q + 1) * (TB // 2)],
                            y_sb[:, hsl],
                        )
                else:
                    nc.vector.tensor_copy(y_sb[:], psum_y[:])
                    nc.sync.dma_start(
                        yT_d[h * P:(h + 1) * P, tb * TB:(tb + 1) * TB], y_sb[:]
                    )
                sums_sb = y_pool.tile([1, TB], f32, tag="sums", name="sums_sb")
                nc.vector.tensor_copy(sums_sb[:], psum_sum[0:1, :])
                nc.sync.dma_start(
                    sums_d[h * TT + tb:h * TT + tb + 1, :], sums_sb[0:1, :]
                )

            # ---- software pipeline: proj(tt) overlapped with attn(tb=tt-1) ----
            GA, GB = [0, 1, 2], [3, 4, 5]
            # step 0: no attention to overlap, so run all 6 j-blocks at once
            # (borrowing the idle attention PSUM banks) — this halves the
            # DMA demand rate per unit of PE work and avoids supply stalls
            ps0 = (
                [psproj_pool.tile([P, TB], f32, tag="proj", name="proj_ps")
                 for _ in range(3)]
                + [psacc_pool.tile([P, TB], f32, tag="acc", name="proj_ps")
                   for _ in range(2)]
                + [pss_pool.tile([P, TB], f32, tag="s", name="proj_ps")]
            )
            proj_mms(0, GA + GB, ps0)
            proj_post(0, GA + GB, ps0)
            for cq in range(4):
                xt_tiles.pop((0, cq), None)
            for step in range(1, TT + 1):
                tt = step if step < TT else None
                tb = step - 1
                attn_head(tb, 0, False)
                attn_head(tb, 1, False)
                if tt is not None:
                    psA = [psproj_pool.tile([P, TB], f32, tag="proj",
                                            name="proj_ps") for _ in GA]
                    proj_mms(tt, GA, psA)
                attn_head(tb, 2, False)
                if tt is not None:
                    proj_post(tt, GA, psA)
                attn_head(tb, 3, False)
                if tt is not None:
                    psB = [psproj_pool.tile([P, TB], f32, tag="proj",
                                            name="proj_ps") for _ in GB]
                    proj_mms(tt, GB, psB)
                    proj_post(tt, GB, psB)
                    # xt tiles for this tt are dead now
                    for cq in range(4):
                        xt_tiles.pop((tt, cq), None)

    nc.compile()
    _cache["nc"] = nc
    return nc


def _host_prep(x, w_qkv, freqs_cos, freqs_sin):
    """Build per-core input maps (numpy, cheap)."""
    x = np.asarray(x, dtype=np.float32)
    w_qkv = np.asarray(w_qkv, dtype=np.float32)
    freqs_cos = np.asarray(freqs_cos, dtype=np.float32)
    freqs_sin = np.asarray(freqs_sin, dtype=np.float32)
    bf = ml_dtypes.bfloat16

    perm = np.concatenate([np.arange(0, HD, 2), np.arange(1, HD, 2)])

    xTs = []
    for b in range(B):
        # [C, T] -> [P, cq, tt, ci, TB]
        xt = x[b].T.reshape(4, 4, P, TT, TB).transpose(2, 0, 3, 1, 4)
        xTs.append(np.ascontiguousarray(xt).astype(bf))

    cosT = freqs_cos.T  # [64, T]
    sinT = freqs_sin.T
    CCh = np.concatenate([cosT, cosT], axis=0).astype(bf)
    SS2 = np.concatenate([-sinT, sinT], axis=0).astype(bf)
    tri = np.triu(np.ones((P, P), dtype=np.float32)).astype(bf)
    ones = np.ones((P, P), dtype=bf)
    ident = np.eye(P, dtype=np.float32).astype(bf)

    in_maps = []
    for core in range(NCORES):
        b, kv = divmod(core, KV)
        blocks = []
        for r in range(NREP):
            hrow = (kv * NREP + r) * HD
            blocks.append(w_qkv[hrow:hrow + HD][perm])
        blocks.append(w_qkv[H * HD + kv * HD:H * HD + (kv + 1) * HD][perm])
        blocks.append(
            w_qkv[(H + KV) * HD + kv * HD:(H + KV) * HD + (kv + 1) * HD]
        )
        w_shard = np.concatenate(blocks, axis=0)  # [768, C]
        # [C, 768] -> [P, wq, ci, 768]
        wT = w_shard.T.reshape(4, 4, P, NB * P).transpose(2, 0, 1, 3)
        wT = np.ascontiguousarray(wT).astype(bf)
        in_maps.append({
            "xT": xTs[b],
            "wT": wT,
            "CC": CCh,
            "SS2": SS2,
            "tri": tri,
            "ones": ones,
            "ident": ident,
        })
    return in_maps


def kernel(x, w_qkv, freqs_cos, freqs_sin):
    nc = _build()
    in_maps = _host_prep(x, w_qkv, freqs_cos, freqs_sin)
    res = run_bass_kernel_spmd(nc, in_maps, list(range(NCORES)), trace=TRACE)
    _cache["last_res"] = res

    y = np.empty((B, T, C), dtype=np.float32)
    for core in range(NCORES):
        b, kv = divmod(core, KV)
        yT = res.results[core]["yT"].astype(np.float32)  # [NREP*P, T] unnorm.
        sums = res.results[core]["sums"].reshape(NREP, T)  # per (h, t)
        yT = yT.reshape(NREP, P, T) / sums[:, None, :]
        y[b, :, kv * NREP * HD:(kv + 1) * NREP * HD] = (
            yT.reshape(NREP * P, T).T
        )
    return y
